# revision 1
# baseline (speedup 1.0000x reference)
"""Trainium2 Bass kernel for nn_CriticAttention (8-core data-parallel).

Math (per reference.py):
  cur  = state[:, ai, :]                       # [B, D]
  s_enc = leaky(bn(cur, axes=0) @ Ws + bs)     # [B, Hid]
  others = state minus agent ai                # [B, A-1, D]
  sa_enc = leaky(bn(others, axes=(0,1)) @ Wc + bc)
  k = einsum('ban,hnd->bhad', sa_enc, Wk)
  v = leaky(einsum('ban,hnd->bhad', sa_enc, Wv))
  q = einsum('bn,hnd->bhd', s_enc, Wq)
  att = softmax(q.k/sqrt(hd)) @ v  -> [B, H*hd]

Mapping:
  - batch sharded over 8 cores (1024 b each), weights replicated.
  - BN folded into the Dense weights: bn(x)@W+b == x@(s*W) + (b - (m*s)@W),
    with global (m, s) combined from per-core bn_stats partials via a small
    AllGather + on-chip reduction.
  - state is cast f32->bf16 (SWDGE cast DMA) into a DRAM scratch laid out
    chunk-major, then DMA-xbar-transposed into xT tiles [128 d, 2048 (a,b)].
  - All big matmuls run on PE in bf16 (PE rate is dtype-independent):
      sa_encT[n, rows] = Wc'^T @ xT   (PSUM drained via ScalarE Prelu+bias)
      k/v/q[rows, h*d] = sa_encT^T-slices @ Wk2d/Wv2d/Wq2d
    k/v land as [b, a, h*d] so every PSUM drain is a contiguous 512-wide op.
  - attention per 128-b chunk on VectorE with b on partitions.
"""

import os
import sys

import numpy as np

if "/opt/trn_rl_repo" not in sys.path:
    sys.path.insert(0, "/opt/trn_rl_repo")

NCORES = 8
B, A, D, Hid, H, HD = 8192, 16, 256, 512, 8, 64
BL = B // NCORES          # batch per core
CB = 128                  # batch per chunk
NCH = BL // CB            # chunks per core
ROWS = A * CB             # rows (a-major) per chunk
NT = Hid // 128           # Hid partition tiles
DT = D // 128             # D partition tiles
AO = A - 1                # number of "other" agents
EPS = 1e-3
ALPHA = 0.3

_CACHE = {}


def _col_ranges(ai, width=512):
    """Column ranges (start, len) over the (a-major, b) 2048-wide chunk that
    cover all agents except `ai`, each piece <= width."""
    out = []
    for lo, hi in ((0, ai * CB), ((ai + 1) * CB, A * CB)):
        c = lo
        while c < hi:
            n = min(width, hi - c)
            out.append((c, n))
            c += n
    return out


def _build(ai: int):
    if ai in _CACHE:
        return _CACHE[ai]

    import concourse.bass as bass
    import concourse.tile as tile
    from concourse import bacc, mybir

    f32 = mybir.dt.float32
    bf16 = mybir.dt.bfloat16
    Alu = mybir.AluOpType
    Act = mybir.ActivationFunctionType

    nc = bacc.Bacc("TRN2", target_bir_lowering=False, debug=False,
                   num_devices=NCORES, name="critic_attention")

    state = nc.dram_tensor("state", [BL, A, D], f32, kind="ExternalInput")
    Ws_d = nc.dram_tensor("Ws", [D, Hid], f32, kind="ExternalInput")
    bs_d = nc.dram_tensor("bs", [Hid], f32, kind="ExternalInput")
    Wc_d = nc.dram_tensor("Wc", [D, Hid], f32, kind="ExternalInput")
    bc_d = nc.dram_tensor("bc", [Hid], f32, kind="ExternalInput")
    Wk_d = nc.dram_tensor("Wk", [H, Hid, HD], f32, kind="ExternalInput")
    Wq_d = nc.dram_tensor("Wq", [H, Hid, HD], f32, kind="ExternalInput")
    Wv_d = nc.dram_tensor("Wv", [H, Hid, HD], f32, kind="ExternalInput")
    out_d = nc.dram_tensor("out", [BL, H * HD], f32, kind="ExternalOutput")

    ranges = _col_ranges(ai)
    nsub = len(ranges)
    CGRP = 2                      # chunks per cast DMA group

    with tile.TileContext(nc) as tc:
        with (
            tc.tile_pool(name="consts", bufs=1) as consts,
            tc.tile_pool(name="dram", bufs=1, space="DRAM") as dram,
            tc.tile_pool(name="xa_pool", bufs=3) as xa_pool,
            tc.tile_pool(name="xb_pool", bufs=2) as xb_pool,
            tc.tile_pool(name="sa_pool", bufs=2) as sa_pool,
            tc.tile_pool(name="at_pool", bufs=2) as at_pool,
            tc.tile_pool(name="psum", bufs=2, space="PSUM") as psum,
        ):
            # ---------------- constants / weights ----------------
            Wc32 = consts.tile([128, DT, Hid], f32)
            Ws32 = consts.tile([128, DT, Hid], f32)
            for dt in range(DT):
                nc.sync.dma_start(Wc32[:, dt, :], Wc_d[dt * 128:(dt + 1) * 128, :])
                nc.sync.dma_start(Ws32[:, dt, :], Ws_d[dt * 128:(dt + 1) * 128, :])

            Wk2 = consts.tile([128, NT, H * HD], bf16)
            Wq2 = consts.tile([128, NT, H * HD], bf16)
            Wv2 = consts.tile([128, NT, H * HD], bf16)
            for w_d, w_sb in ((Wk_d, Wk2), (Wq_d, Wq2), (Wv_d, Wv2)):
                for kt in range(NT):
                    src = w_d[:, kt * 128:(kt + 1) * 128, :].rearrange("h p d -> p h d")
                    nc.gpsimd.dma_start(w_sb[:, kt, :].rearrange("p (h d) -> p h d", h=H), src)

            bcT = consts.tile([128, NT], f32)
            bsT = consts.tile([128, NT], f32)
            with nc.allow_non_contiguous_dma("tiny bias transpose loads"):
                nc.gpsimd.dma_start(bcT[:, :], bc_d.rearrange("(j p) -> p j", p=128))
                nc.gpsimd.dma_start(bsT[:, :], bs_d.rearrange("(j p) -> p j", p=128))

            # bf16 scratch for transposed loads: [chunk][a][b][d]
            xbf = dram.tile([NCH, A, CB, D], bf16)

            # ---------------- phase A: cast, transpose, stats ----------------
            for t in range(NCH):
                src = state[t * CB:(t + 1) * CB, :, :].rearrange("b a d -> a b d")
                nc.gpsimd.dma_start(xbf[t], src)

            stato = consts.tile([128, DT, NCH, nsub, 6], f32)
            statc = consts.tile([128, DT, NCH, 6], f32)
            for t in range(NCH):
                flat = xbf[t].rearrange("a b d -> (a b) d")
                for dt in range(DT):
                    xa = xa_pool.tile([128, ROWS], bf16, tag="xa", name=f"xa_{t}_{dt}")
                    nc.sync.dma_start(xa[:, :], flat[:, dt * 128:(dt + 1) * 128],
                                      transpose=True)
                    for i, (c0, cn) in enumerate(ranges):
                        nc.vector.bn_stats(stato[:, dt, t, i, :], xa[:, c0:c0 + cn])
                    nc.vector.bn_stats(statc[:, dt, t, :],
                                       xa[:, ai * CB:(ai + 1) * CB])

            aggo = consts.tile([128, DT, 2], f32)
            aggc = consts.tile([128, DT, 2], f32)
            for dt in range(DT):
                nc.vector.bn_aggr(aggo[:, dt, :], stato[:, dt])
                nc.vector.bn_aggr(aggc[:, dt, :], statc[:, dt])

            # pack (mean, E[x^2]) per (dt, grp) -> [128, 8]; AllGather over the
            # 8 cores (partition-axis stacking) then on-chip sum + /NCORES.
            cc_sb = consts.tile([128, DT, 2, 2], f32)
            for dt in range(DT):
                for g, agg in enumerate((aggo, aggc)):
                    m = agg[:, dt, 0:1]
                    v = agg[:, dt, 1:2]
                    nc.vector.tensor_copy(cc_sb[:, dt, g, 0:1], m)
                    # E2 = m*m + v  (scalar slot broadcasts m per partition)
                    nc.vector.scalar_tensor_tensor(
                        cc_sb[:, dt, g, 1:2], in0=m, scalar=m, in1=v,
                        op0=Alu.mult, op1=Alu.add)

            cc_in = dram.tile([128, DT * 4], f32)
            cc_out = dram.tile([128 * NCORES, DT * 4], f32, addr_space="Shared")
            nc.gpsimd.dma_start(cc_in[:, :], cc_sb.rearrange("p a b c -> p (a b c)"))
            nc.gpsimd.collective_compute(
                "AllGather", Alu.bypass,
                replica_groups=[list(range(NCORES))],
                ins=[cc_in.opt()], outs=[cc_out.opt()])
            ccg = consts.tile([128, NCORES, DT * 4], f32)
            with nc.allow_non_contiguous_dma("tiny stats gather load"):
                nc.gpsimd.dma_start(
                    ccg[:, :, :],
                    cc_out.rearrange("(r p) v -> p r v", p=128))
            ccr = consts.tile([128, DT, 2, 2], f32)
            # sum over ranks (innermost after view) then scale by 1/NCORES
            nc.vector.tensor_reduce(
                ccr.rearrange("p a b c -> p (a b c)"),
                ccg.rearrange("p r v -> p v r"),
                axis=mybir.AxisListType.X, op=Alu.add)

            gm = consts.tile([128, DT, 2], f32)     # mean   per (dt, grp)
            ge = consts.tile([128, DT, 2], f32)     # E[x^2] per (dt, grp)
            nc.vector.tensor_scalar_mul(gm.rearrange("p a b -> p (a b)"),
                                        ccr[:, :, :, 0].rearrange("p a b -> p (a b)"),
                                        1.0 / NCORES)
            nc.vector.tensor_scalar_mul(ge.rearrange("p a b -> p (a b)"),
                                        ccr[:, :, :, 1].rearrange("p a b -> p (a b)"),
                                        1.0 / NCORES)
            var4 = consts.tile([128, DT, 2], f32)
            mm4 = consts.tile([128, DT, 2], f32)
            nc.vector.tensor_mul(mm4[:, :, :], gm[:, :, :], gm[:, :, :])
            nc.vector.tensor_sub(var4[:, :, :], ge[:, :, :], mm4[:, :, :])
            eps_t = consts.tile([128, 1], f32)
            nc.vector.memset(eps_t[:, :], float(EPS))
            ln4 = consts.tile([128, DT, 2], f32)
            nc.scalar.activation(ln4.rearrange("p a b -> p (a b)"),
                                 var4.rearrange("p a b -> p (a b)"),
                                 Act.Ln, bias=eps_t[:, :])
            s4 = consts.tile([128, DT, 2], f32)     # rsqrt(var+eps)
            nc.scalar.activation(s4.rearrange("p a b -> p (a b)"),
                                 ln4.rearrange("p a b -> p (a b)"),
                                 Act.Exp, scale=-0.5)
            nms4 = consts.tile([128, DT, 2], f32)   # -mean * s
            nc.vector.scalar_tensor_tensor(
                nms4.rearrange("p a b -> p (a b)"),
                in0=gm.rearrange("p a b -> p (a b)"), scalar=-1.0,
                in1=s4.rearrange("p a b -> p (a b)"),
                op0=Alu.mult, op1=Alu.mult)

            # ---------------- fold BN into weights ----------------
            Wcb = consts.tile([128, DT, Hid], bf16)
            Wsb = consts.tile([128, DT, Hid], bf16)
            for dt in range(DT):
                nc.vector.tensor_scalar_mul(Wcb[:, dt, :], Wc32[:, dt, :],
                                            s4[:, dt, 0:1])
                nc.vector.tensor_scalar_mul(Wsb[:, dt, :], Ws32[:, dt, :],
                                            s4[:, dt, 1:2])
            biasC = consts.tile([128, NT], f32)
            biasS = consts.tile([128, NT], f32)
            for j in range(NT):
                for bias_t, w32, base_t, g in ((biasC, Wc32, bcT, 0),
                                               (biasS, Ws32, bsT, 1)):
                    ps = psum.tile([128, 512], f32, tag="enc", name=f"psf_{j}_{g}")
                    for dt in range(DT):
                        nc.tensor.matmul(ps[:, 0:1],
                                         lhsT=w32[:, dt, j * 128:(j + 1) * 128],
                                         rhs=nms4[:, dt, g:g + 1],
                                         start=(dt == 0), stop=(dt == DT - 1))
                    nc.scalar.activation(bias_t[:, j:j + 1], ps[:, 0:1],
                                         Act.Identity, bias=base_t[:, j:j + 1])

            # ---------------- phase B: encoders, K/V/Q, attention ----------------
            for t in range(NCH):
                flat = xbf[t].rearrange("a b d -> (a b) d")
                xb = []
                for dt in range(DT):
                    xt = xb_pool.tile([128, ROWS], bf16, tag=f"xb{dt}",
                                      name=f"xb_{t}_{dt}")
                    nc.sync.dma_start(xt[:, :], flat[:, dt * 128:(dt + 1) * 128],
                                      transpose=True)
                    xb.append(xt)

                # encoders -> sa_encT (others, a-slot compacted), sq_encT (cur)
                saT = sa_pool.tile([128, NT, AO * CB], bf16, tag="saT",
                                   name=f"saT_{t}")
                sqT = sa_pool.tile([128, NT, CB], bf16, tag="sqT", name=f"sqT_{t}")
                for j in range(NT):
                    for (c0, cn) in ranges:
                        ps = psum.tile([128, 512], f32, tag="enc",
                                       name=f"pse_{t}_{j}_{c0}")
                        for dt in range(DT):
                            nc.tensor.matmul(ps[:, :cn],
                                             lhsT=Wcb[:, dt, j * 128:(j + 1) * 128],
                                             rhs=xb[dt][:, c0:c0 + cn],
                                             start=(dt == 0), stop=(dt == DT - 1))
                        c0p = c0 if c0 < ai * CB else c0 - CB
                        nc.scalar.activation(saT[:, j, c0p:c0p + cn], ps[:, :cn],
                                             Act.Prelu, bias=biasC[:, j:j + 1],
                                             alpha=ALPHA)
                    psq = psum.tile([128, CB], f32, tag="encq", name=f"psq_{t}_{j}")
                    for dt in range(DT):
                        nc.tensor.matmul(psq[:, :],
                                         lhsT=Wsb[:, dt, j * 128:(j + 1) * 128],
                                         rhs=xb[dt][:, ai * CB:(ai + 1) * CB],
                                         start=(dt == 0), stop=(dt == DT - 1))
                    nc.scalar.activation(sqT[:, j, :], psq[:, :], Act.Prelu,
                                         bias=biasS[:, j:j + 1], alpha=ALPHA)

                # Q
                psqq = psum.tile([128, H * HD], f32, tag="kvq", bufs=4,
                                 name=f"psqq_{t}")
                for kt in range(NT):
                    nc.tensor.matmul(psqq[:, :], lhsT=sqT[:, kt, :],
                                     rhs=Wq2[:, kt, :],
                                     start=(kt == 0), stop=(kt == NT - 1))
                q_all = at_pool.tile([128, H * HD], bf16, tag="q", name=f"q_{t}")
                nc.scalar.copy(q_all[:, :], psqq[:, :])

                # K, V for each other-agent; [b, a, (h d)] so drains are
                # contiguous 512-wide PSUM->SBUF ops.
                k_all = at_pool.tile([128, AO, H * HD], bf16, tag="k", name=f"k_{t}")
                v_all = at_pool.tile([128, AO, H * HD], bf16, tag="v", name=f"v_{t}")
                for ae in range(AO):
                    psk = psum.tile([128, H * HD], f32, tag="kvq", bufs=4,
                                    name=f"psk_{t}_{ae}")
                    for kt in range(NT):
                        nc.tensor.matmul(psk[:, :],
                                         lhsT=saT[:, kt, ae * CB:(ae + 1) * CB],
                                         rhs=Wk2[:, kt, :],
                                         start=(kt == 0), stop=(kt == NT - 1))
                    nc.scalar.copy(k_all[:, ae, :], psk[:, :])
                    psv = psum.tile([128, H * HD], f32, tag="kvq", bufs=4,
                                    name=f"psv_{t}_{ae}")
                    for kt in range(NT):
                        nc.tensor.matmul(psv[:, :],
                                         lhsT=saT[:, kt, ae * CB:(ae + 1) * CB],
                                         rhs=Wv2[:, kt, :],
                                         start=(kt == 0), stop=(kt == NT - 1))
                    nc.scalar.activation(v_all[:, ae, :], psv[:, :],
                                         Act.Prelu, alpha=ALPHA)

                # scores = sum_d q*k  -> [128, H, AO]
                prod = at_pool.tile([128, H * AO * HD], bf16, tag="prod",
                                    name=f"prod_{t}")
                p3 = prod.rearrange("p (h a d) -> p h a d", h=H, a=AO)
                k_v = k_all.rearrange("p a (h d) -> p h a d", h=H)
                q_b = q_all.rearrange("p (h d) -> p h d", h=H) \
                           .unsqueeze(2).broadcast_to([128, H, AO, HD])
                nc.vector.tensor_mul(p3, k_v, q_b)
                scores = at_pool.tile([128, H, AO], f32, tag="scores",
                                      name=f"scores_{t}")
                nc.vector.tensor_reduce(
                    scores[:, :, :],
                    prod.rearrange("p (g d) -> p g d", d=HD),
                    axis=mybir.AxisListType.X, op=Alu.add)

                # softmax (no max-subtraction: |scores/8| << 1), with 1/sqrt(hd)
                e15 = at_pool.tile([128, H, AO], bf16, tag="e15", name=f"e15_{t}")
                nc.scalar.activation(e15[:, :, :], scores[:, :, :], Act.Exp,
                                     scale=1.0 / float(np.sqrt(HD)))
                sums = at_pool.tile([128, H], f32, tag="sums", name=f"sums_{t}")
                nc.vector.tensor_reduce(sums[:, :], e15[:, :, :],
                                        axis=mybir.AxisListType.X, op=Alu.add)
                rinv = at_pool.tile([128, H], f32, tag="rinv", name=f"rinv_{t}")
                nc.vector.reciprocal(rinv[:, :], sums[:, :])

                # att_raw = sum_a e*v ; out = att_raw * rinv
                v_v = v_all.rearrange("p a (h d) -> p h a d", h=H)
                e_b = e15.unsqueeze(3).broadcast_to([128, H, AO, HD])
                nc.vector.tensor_mul(p3, v_v, e_b)
                att_raw = at_pool.tile([128, H * HD], f32, tag="att",
                                       name=f"att_{t}")
                nc.vector.tensor_reduce(
                    att_raw.rearrange("p (h d) -> p h d", h=H),
                    prod.rearrange("p (h a d) -> p h d a", h=H, a=AO),
                    axis=mybir.AxisListType.X, op=Alu.add)
                out_t = at_pool.tile([128, H * HD], f32, tag="out", name=f"out_{t}")
                r_b = rinv.unsqueeze(2).broadcast_to([128, H, HD])
                nc.vector.tensor_mul(out_t.rearrange("p (h d) -> p h d", h=H),
                                     att_raw.rearrange("p (h d) -> p h d", h=H),
                                     r_b)
                nc.sync.dma_start(out_d[t * CB:(t + 1) * CB, :], out_t[:, :])

    nc.compile()
    _CACHE[ai] = nc
    return nc


def _run(inputs, trace=False, **kwargs):
    from concourse.bass_utils import run_bass_kernel_spmd

    state = np.ascontiguousarray(np.asarray(inputs["state"], dtype=np.float32))
    ai = int(np.asarray(inputs["agent_index"]))
    arrs = {}
    for name in ("Ws", "bs", "Wc", "bc", "Wk", "Wq", "Wv"):
        arrs[name] = np.ascontiguousarray(np.asarray(inputs[name], dtype=np.float32))

    nc = _build(ai)
    in_maps = []
    for c in range(NCORES):
        m = {"state": np.ascontiguousarray(state[c * BL:(c + 1) * BL])}
        m.update(arrs)
        in_maps.append(m)
    res = run_bass_kernel_spmd(nc, in_maps, core_ids=list(range(NCORES)),
                               trace=trace, **kwargs)
    out = np.concatenate([r["out"] for r in res.results], axis=0).astype(np.float32)
    return out, res


def kernel(**inputs) -> np.ndarray:
    out, _ = _run(inputs, trace=False)
    return out

